# revision 22
# baseline (speedup 1.0000x reference)
"""Trainium2 Bass kernel for the EncoderSAE problem.

Computes, for x (4096, 1024), W_enc (32768, 1024), W_dec (1024, 32768):
    feats = relu(x @ W_enc.T)
    vals, idx = top_k(feats, 64)
    sparse = scatter(vals at idx)            # (4096, 32768)
    reconstructed = sparse @ W_dec.T          # (4096, 1024)
    l0 = mean(count(sparse > 0, axis=1))

Strategy: data-parallel over the batch on 8 NeuronCores (512 rows each).
Per core, encode is a fp32 PE matmul chunked 512 features at a time
(encoder weights streamed once, chunk-outer / row-block-inner).  Top-64
per row is computed exactly via per-chunk top-8 candidates (DVE max8) +
8 rounds of max8/match_replace over the 512 candidates; the 64th value
is a per-row threshold used to mask the feats stream into the sparse
output.  Decode gathers the 64 selected decoder rows per batch row with
indirect DMA and accumulates them with per-partition-scalar multiply-add
on the vector engine.  Rare exact-tie/chunk-overflow rows are detected
on-device (cnt/susp outputs) and repaired exactly on host.
"""

import sys

if "/opt/trn_rl_repo" not in sys.path:
    sys.path.insert(0, "/opt/trn_rl_repo")

import numpy as np

import concourse.bass as bass
import concourse.mybir as mybir
import concourse.tile as tile
from concourse import bacc
from concourse.bass import IndirectOffsetOnAxis
from concourse.bass_utils import run_bass_kernel_spmd

# Problem constants
INPUT_DIM = 1024
DICT_SIZE = 32768
K = 64
BATCH = 4096
N_CORES = 8

P = 128  # partitions
CHUNK = 512  # features per encode chunk


def build_kernel(
    bpc: int = BATCH // N_CORES,  # batch rows per core
    dim: int = INPUT_DIM,
    dict_size: int = DICT_SIZE,
):
    n_blocks = bpc // P
    n_chunks = dict_size // CHUNK
    n_k = dim // P  # contraction tiles
    f32 = mybir.dt.float32
    u32 = mybir.dt.uint32
    u16 = mybir.dt.uint16

    nc = bacc.Bacc("TRN2", target_bir_lowering=False, num_devices=N_CORES)

    xt = nc.dram_tensor("xt", [dim, bpc], f32, kind="ExternalInput")
    wencT = nc.dram_tensor("wencT", [dim, dict_size], f32, kind="ExternalInput")
    wdecT = nc.dram_tensor("wdecT", [dict_size, dim], f32, kind="ExternalInput")
    diagk = nc.dram_tensor("diagk", [P, K * 16], f32, kind="ExternalInput")
    rowmul = nc.dram_tensor("rowmul", [P, 1], f32, kind="ExternalInput")

    # declared flat so the indirect scatter can use axis-0 indirection
    sparse = nc.dram_tensor(
        "sparse", [bpc * dict_size, 1], f32, kind="ExternalOutput"
    )
    recon = nc.dram_tensor("recon", [bpc, dim], f32, kind="ExternalOutput")
    vals_o = nc.dram_tensor("vals", [bpc, K], f32, kind="ExternalOutput")
    oidx_o = nc.dram_tensor("oidx", [bpc, K], u32, kind="ExternalOutput")
    cnt_o = nc.dram_tensor("cnt", [bpc, 1], f32, kind="ExternalOutput")
    susp_o = nc.dram_tensor("susp", [bpc, 1], f32, kind="ExternalOutput")

    Relu = mybir.ActivationFunctionType.Relu
    Alu = mybir.AluOpType

    with tile.TileContext(nc) as tc:
        with (
            tc.tile_pool(name="persist", bufs=1) as persist,
            tc.tile_pool(name="wpool", bufs=2) as wpool,
            tc.tile_pool(name="fpool", bufs=3) as fpool,
            tc.tile_pool(name="spool", bufs=2) as spool,
            tc.tile_pool(name="gpool", bufs=4) as gpool,
            tc.tile_pool(name="psum", bufs=2, space="PSUM") as pspool,
        ):
            # xt resident: [128, n_k, bpc] (d-tile on partitions)
            xt_sb = persist.tile([P, n_k, bpc], f32)
            for k in range(n_k):
                nc.sync.dma_start(xt_sb[:, k, :], xt[k * P : (k + 1) * P, :])

            # per-block candidate arrays (top-8 of each chunk + global idx)
            top8 = [persist.tile([P, n_chunks * 8], f32, tag=f"top8_{b}",
                                 name=f"top8_{b}") for b in range(n_blocks)]
            idxg = [persist.tile([P, n_chunks * 8], u32, tag=f"idxg_{b}",
                                 name=f"idxg_{b}") for b in range(n_blocks)]

            # constant mask for diagonal extraction of the 16-partition-shared
            # indirect_copy gather: diagK[p, j*16+s] = (s == p%16)
            diagK = persist.tile([P, K * 16], f32)
            nc.sync.dma_start(diagK[:], diagk[:])
            # rowmul[p] = p * dict_size, for flattening scatter indices
            rowm = persist.tile([P, 1], f32)
            nc.sync.dma_start(rowm[:], rowmul[:])

            # ---------------- Phase A: encode + per-chunk candidates ----
            for c in range(n_chunks):
                w_sb = wpool.tile([P, n_k, CHUNK], f32, tag="w")
                for k in range(n_k):
                    nc.sync.dma_start(
                        w_sb[:, k, :],
                        wencT[k * P : (k + 1) * P, c * CHUNK : (c + 1) * CHUNK],
                    )
                for b in range(n_blocks):
                    ps = pspool.tile([P, CHUNK], f32, tag="enc")
                    for k in range(n_k):
                        nc.tensor.matmul(
                            out=ps[:],
                            lhsT=xt_sb[:, k, b * P : (b + 1) * P],
                            rhs=w_sb[:, k, :],
                            start=(k == 0),
                            stop=(k == n_k - 1),
                        )
                    fch = fpool.tile([P, CHUNK], f32, tag="fch")
                    nc.scalar.activation(fch[:], ps[:], Relu)
                    t8 = top8[b][:, c * 8 : (c + 1) * 8]
                    ig = idxg[b][:, c * 8 : (c + 1) * 8]
                    nc.vector.max(out=t8, in_=fch[:])
                    nc.vector.max_index(out=ig, in_max=t8, in_values=fch[:])
                    if c > 0:
                        # convert chunk-local index to global feature index
                        nc.vector.tensor_scalar_add(ig, ig, float(c * CHUNK))

            # ------------- Phases B/C/D per row-block -------------------
            for b in range(n_blocks):
                bs = slice(b * P, (b + 1) * P)
                ncand = n_chunks * 8

                # B: top-64 of candidates
                work = spool.tile([P, ncand], f32, tag="work")
                nc.vector.tensor_copy(work[:], top8[b][:])
                vals_sb = spool.tile([P, K], f32, tag="vals")
                cidx_sb = spool.tile([P, K], u16, tag="cidx")
                for r in range(K // 8):
                    m8 = vals_sb[:, r * 8 : (r + 1) * 8]
                    nc.vector.max(out=m8, in_=work[:])
                    nc.vector.max_index(
                        out=cidx_sb[:, r * 8 : (r + 1) * 8], in_max=m8, in_values=work[:]
                    )
                    nc.vector.match_replace(
                        out=work[:], in_to_replace=m8, in_values=work[:], imm_value=-1.0
                    )
                t_ap = vals_sb[:, K - 1 : K]

                # exactness checks: cnt = #candidates >= T (must be 64);
                # susp = #chunks whose 8th max >= T (must be 0)
                chk = spool.tile([P, ncand], f32, tag="chk")
                cnt_sb = spool.tile([P, 1], f32, tag="cnt")
                nc.vector.scalar_tensor_tensor(
                    out=chk[:], in0=top8[b][:], scalar=t_ap, in1=top8[b][:],
                    op0=Alu.is_ge, op1=Alu.bypass, accum_out=cnt_sb[:],
                )
                chk8 = spool.tile([P, n_chunks], f32, tag="chk8")
                susp_sb = spool.tile([P, 1], f32, tag="susp")
                eighth = top8[b][:, 7:ncand:8]
                nc.vector.scalar_tensor_tensor(
                    out=chk8[:], in0=eighth, scalar=t_ap, in1=eighth,
                    op0=Alu.is_ge, op1=Alu.bypass, accum_out=susp_sb[:],
                )

                # original feature indices of the selected 64.  indirect_copy
                # shares one interleaved index list per 16-partition group, so
                # gather [P, K*16] and extract the diagonal (s == p%16) where
                # each row's own lookup lands.
                gath = spool.tile([P, K * 16], u32, tag="gath")
                nc.gpsimd.indirect_copy(
                    out=gath[:], data=idxg[b][:], idxs=cidx_sb[:],
                    i_know_ap_gather_is_preferred=True,
                )
                gathf = spool.tile([P, K * 16], f32, tag="gathf")
                nc.vector.tensor_copy(gathf[:], gath[:])
                prodk = spool.tile([P, K * 16], f32, tag="prodk")
                nc.vector.tensor_tensor(
                    out=prodk[:], in0=gathf[:], in1=diagK[:], op=Alu.mult
                )
                oidxf = spool.tile([P, K], f32, tag="oidxf")
                nc.vector.reduce_sum(
                    oidxf[:],
                    prodk[:].rearrange("p (k s) -> p k s", s=16),
                    axis=mybir.AxisListType.X,
                )
                oidx_sb = spool.tile([P, K], u32, tag="oidx")
                nc.vector.tensor_copy(oidx_sb[:], oidxf[:])

                nc.sync.dma_start(vals_o[bs, :], vals_sb[:])
                nc.sync.dma_start(oidx_o[bs, :], oidx_sb[:])
                nc.sync.dma_start(cnt_o[bs, :], cnt_sb[:])
                nc.sync.dma_start(susp_o[bs, :], susp_sb[:])

                # C: scatter the 64 exact values into the (pre-zeroed) sparse
                # output.  flat index = p*dict_size + oidx, block offset via
                # element_offset.
                flatf = spool.tile([P, K], f32, tag="flatf")
                nc.vector.scalar_tensor_tensor(
                    out=flatf[:], in0=oidxf[:], scalar=rowm[:], in1=oidxf[:],
                    op0=Alu.add, op1=Alu.bypass,
                )
                flatu = spool.tile([P, K], u32, tag="flatu")
                nc.vector.tensor_copy(flatu[:], flatf[:])
                # HW consumes one dynamic index per partition-row per
                # instruction, so scatter one of the 64 columns at a time.
                for g in range(K):
                    nc.gpsimd.indirect_dma_start(
                        out=sparse.ap(),
                        out_offset=IndirectOffsetOnAxis(
                            ap=flatu[:, g : g + 1], axis=0
                        ),
                        in_=vals_sb[:, g : g + 1],
                        in_offset=None,
                        element_offset=b * P * dict_size,
                    )

                # D: decode via indirect row gathers + per-row MAC
                acc = spool.tile([P, dim], f32, tag="acc")
                nc.vector.memset(acc[:], 0.0)
                for g in range(K):
                    gt = gpool.tile([P, dim], f32, tag="gt")
                    nc.gpsimd.indirect_dma_start(
                        out=gt[:],
                        out_offset=None,
                        in_=wdecT.ap(),
                        in_offset=IndirectOffsetOnAxis(
                            ap=oidx_sb[:, g : g + 1], axis=0
                        ),
                    )
                    nc.vector.scalar_tensor_tensor(
                        out=acc[:], in0=gt[:], scalar=vals_sb[:, g : g + 1],
                        in1=acc[:], op0=Alu.mult, op1=Alu.add,
                    )
                nc.sync.dma_start(recon[bs, :], acc[:])

    nc.compile()
    return nc


_NC_CACHE = {}


def _get_kernel(bpc, dim, dict_size):
    key = (bpc, dim, dict_size)
    if key not in _NC_CACHE:
        _NC_CACHE[key] = build_kernel(bpc, dim, dict_size)
    return _NC_CACHE[key]


def _host_topk_row(feats_row, k=K):
    """jax.lax.top_k semantics: descending values, ties broken by lower index."""
    idx = np.argsort(-feats_row, kind="stable")[:k]
    return feats_row[idx], idx


def _repair_rows(rows, x, W_enc, W_dec, sparse, recon, vals_all):
    for r in rows:
        feats_row = np.maximum(x[r] @ W_enc.T, 0.0).astype(np.float32)
        v, i = _host_topk_row(feats_row)
        srow = np.zeros(W_enc.shape[0], dtype=np.float32)
        srow[i] = v
        sparse[r] = srow
        recon[r] = (W_dec[:, i] * v[None, :]).sum(axis=1).astype(np.float32)
        vals_all[r] = v


LAST_EXEC_NS = None
LAST_RESULTS = None


def kernel(x, W_enc, W_dec, _trace=False):
    global LAST_EXEC_NS, LAST_RESULTS
    x = np.asarray(x, dtype=np.float32)
    W_enc = np.asarray(W_enc, dtype=np.float32)
    W_dec = np.asarray(W_dec, dtype=np.float32)
    batch, dim = x.shape
    dict_size = W_enc.shape[0]
    bpc = batch // N_CORES

    wencT = np.ascontiguousarray(W_enc.T)
    wdecT = np.ascontiguousarray(W_dec.T)
    pp, ss = np.arange(P)[:, None], np.tile(np.arange(16), K)[None, :]
    diagk = ((pp % 16) == ss).astype(np.float32)
    rowmul = (np.arange(P)[:, None] * dict_size).astype(np.float32)
    in_maps = []
    for c in range(N_CORES):
        xs = x[c * bpc : (c + 1) * bpc]
        in_maps.append(
            {
                "xt": np.ascontiguousarray(xs.T),
                "wencT": wencT,
                "wdecT": wdecT,
                "diagk": diagk,
                "rowmul": rowmul,
            }
        )

    nc = _get_kernel(bpc, dim, dict_size)
    res = run_bass_kernel_spmd(
        nc, in_maps, core_ids=list(range(N_CORES)), trace=_trace
    )
    LAST_EXEC_NS = res.exec_time_ns
    LAST_RESULTS = res

    sparse = np.concatenate(
        [r["sparse"].reshape(bpc, dict_size) for r in res.results], axis=0
    )
    recon = np.concatenate([r["recon"] for r in res.results], axis=0)
    vals_all = np.concatenate([r["vals"] for r in res.results], axis=0)
    oidx_all = np.concatenate([r["oidx"] for r in res.results], axis=0)
    cnt = np.concatenate([r["cnt"] for r in res.results], axis=0)[:, 0]
    susp = np.concatenate([r["susp"] for r in res.results], axis=0)[:, 0]

    # exact repair of rows with boundary ties / chunk overflow / dup values
    bad = (cnt != float(K)) | (susp != 0.0)
    dup = (np.sort(oidx_all, axis=1)[:, 1:] == np.sort(oidx_all, axis=1)[:, :-1]).any(
        axis=1
    )
    bad_rows = np.nonzero(bad | dup)[0]
    if len(bad_rows):
        _repair_rows(bad_rows, x, W_enc, W_dec, sparse, recon, vals_all)

    l0 = np.float32((vals_all > 0).sum(axis=1).mean())
    return recon, sparse, l0


if __name__ == "__main__":
    # tiny shape smoke (build only)
    build_kernel(bpc=128, dim=256, dict_size=2048)
    print("build ok")


# revision 25
# speedup vs baseline: 1.1337x; 1.1337x over previous
"""Trainium2 Bass kernel for the EncoderSAE problem.

Computes, for x (4096, 1024), W_enc (32768, 1024), W_dec (1024, 32768):
    feats = relu(x @ W_enc.T)
    vals, idx = top_k(feats, 64)
    sparse = scatter(vals at idx)            # (4096, 32768)
    reconstructed = sparse @ W_dec.T          # (4096, 1024)
    l0 = mean(count(sparse > 0, axis=1))

Strategy: data-parallel over the batch on 8 NeuronCores (512 rows each).
Per core, rows are processed in 4 blocks of 128 (one per partition).
Block-outer loop: a block's full feature row (128 x 32768 fp32, 16 MB)
stays resident in SBUF while its 64 encode chunks (512 features each,
fp32 PE matmuls) stream through PSUM.  Top-64 per row is computed
exactly from per-chunk top-8 candidates (DVE max8) + 8 rounds of
max8/match_replace over the 512 candidates; the 64th value is a per-row
threshold used to mask the resident feats into the sparse output.
Decode gathers the 64 selected decoder rows per batch row with indirect
DMA and accumulates them with per-partition-scalar multiply-add on the
vector engine.  Rare exact-tie/chunk-overflow rows are detected
on-device (cnt/susp outputs) and repaired exactly on host.
"""

import sys

if "/opt/trn_rl_repo" not in sys.path:
    sys.path.insert(0, "/opt/trn_rl_repo")

import numpy as np

import concourse.bass as bass
import concourse.mybir as mybir
import concourse.tile as tile
from concourse import bacc
from concourse.bass import IndirectOffsetOnAxis
from concourse.bass_utils import run_bass_kernel_spmd

# Problem constants
INPUT_DIM = 1024
DICT_SIZE = 32768
K = 64
BATCH = 4096
N_CORES = 8

P = 128  # partitions
CHUNK = 512  # features per encode chunk


def build_kernel(
    bpc: int = BATCH // N_CORES,  # batch rows per core
    dim: int = INPUT_DIM,
    dict_size: int = DICT_SIZE,
):
    n_blocks = bpc // P
    n_chunks = dict_size // CHUNK
    n_k = dim // P  # contraction tiles
    f32 = mybir.dt.float32
    u32 = mybir.dt.uint32
    u16 = mybir.dt.uint16

    nc = bacc.Bacc("TRN2", target_bir_lowering=False, num_devices=N_CORES)

    xt = nc.dram_tensor("xt", [dim, bpc], f32, kind="ExternalInput")
    wencT = nc.dram_tensor("wencT", [dim, dict_size], f32, kind="ExternalInput")
    wdecT = nc.dram_tensor("wdecT", [dict_size, dim], f32, kind="ExternalInput")
    diagk = nc.dram_tensor("diagk", [P, K * 16], f32, kind="ExternalInput")

    sparse = nc.dram_tensor("sparse", [bpc, dict_size], f32, kind="ExternalOutput")
    recon = nc.dram_tensor("recon", [bpc, dim], f32, kind="ExternalOutput")
    vals_o = nc.dram_tensor("vals", [bpc, K], f32, kind="ExternalOutput")
    oidx_o = nc.dram_tensor("oidx", [bpc, K], u32, kind="ExternalOutput")
    cnt_o = nc.dram_tensor("cnt", [bpc, 1], f32, kind="ExternalOutput")
    susp_o = nc.dram_tensor("susp", [bpc, 1], f32, kind="ExternalOutput")

    Relu = mybir.ActivationFunctionType.Relu
    Alu = mybir.AluOpType
    ncand = n_chunks * 8

    with tile.TileContext(nc) as tc:
        with (
            tc.tile_pool(name="persist", bufs=1) as persist,
            tc.tile_pool(name="xpool", bufs=2) as xpool,
            tc.tile_pool(name="wpool", bufs=10) as wpool,
            tc.tile_pool(name="fpool", bufs=1) as fpool,
            tc.tile_pool(name="cpool", bufs=2) as cpool,
            tc.tile_pool(name="spool", bufs=1) as spool,
            tc.tile_pool(name="opool", bufs=2) as opool,
            tc.tile_pool(name="gpool", bufs=3) as gpool,
            tc.tile_pool(name="psum", bufs=2, space="PSUM") as pspool,
        ):
            # constant mask for diagonal extraction of the 16-partition-shared
            # indirect_copy gather: diagK[p, j*16+s] = (s == p%16)
            diagK = persist.tile([P, K * 16], f32)
            nc.sync.dma_start(diagK[:], diagk[:])

            for b in range(n_blocks):
                bs = slice(b * P, (b + 1) * P)

                # x block, d-tile on partitions: lhsT tiles [128d, 128b]
                xt_sb = xpool.tile([P, n_k, P], f32, tag="xt")
                for k in range(n_k):
                    nc.sync.dma_start(xt_sb[:, k, :], xt[k * P : (k + 1) * P, bs])

                feats = fpool.tile([P, dict_size], f32, tag="feats")
                top8 = cpool.tile([P, ncand], f32, tag="top8")
                idxg = cpool.tile([P, ncand], u32, tag="idxg")

                # ---- A: encode chunks; feats stay resident ----
                for c in range(n_chunks):
                    wk = []
                    for k in range(n_k):
                        w_sb = wpool.tile([P, CHUNK], f32, tag="w", name=f"w_{b}_{c}_{k}")
                        nc.sync.dma_start(
                            w_sb[:],
                            wencT[k * P : (k + 1) * P, c * CHUNK : (c + 1) * CHUNK],
                        )
                        wk.append(w_sb)
                    ps = pspool.tile([P, CHUNK], f32, tag="enc")
                    for k in range(n_k):
                        nc.tensor.matmul(
                            out=ps[:],
                            lhsT=xt_sb[:, k, :],
                            rhs=wk[k][:],
                            start=(k == 0),
                            stop=(k == n_k - 1),
                        )
                    fch = feats[:, c * CHUNK : (c + 1) * CHUNK]
                    nc.scalar.activation(fch, ps[:], Relu)
                    t8 = top8[:, c * 8 : (c + 1) * 8]
                    ig = idxg[:, c * 8 : (c + 1) * 8]
                    nc.vector.max(out=t8, in_=fch)
                    nc.vector.max_index(out=ig, in_max=t8, in_values=fch)
                    if c > 0:
                        # convert chunk-local index to global feature index
                        nc.vector.tensor_scalar_add(ig, ig, float(c * CHUNK))

                # ---- B: top-64 of the 512 candidates ----
                work = spool.tile([P, ncand], f32, tag="work")
                nc.vector.tensor_copy(work[:], top8[:])
                vals_sb = spool.tile([P, K], f32, tag="vals")
                cidx_sb = spool.tile([P, K], u16, tag="cidx")
                for r in range(K // 8):
                    m8 = vals_sb[:, r * 8 : (r + 1) * 8]
                    nc.vector.max(out=m8, in_=work[:])
                    nc.vector.max_index(
                        out=cidx_sb[:, r * 8 : (r + 1) * 8],
                        in_max=m8,
                        in_values=work[:],
                    )
                    nc.vector.match_replace(
                        out=work[:], in_to_replace=m8, in_values=work[:],
                        imm_value=-1.0,
                    )
                t_ap = vals_sb[:, K - 1 : K]

                # exactness checks: cnt = #candidates >= T (must be 64);
                # susp = #chunks whose 8th max >= T (must be 0)
                chk = spool.tile([P, ncand], f32, tag="chk")
                cnt_sb = spool.tile([P, 1], f32, tag="cnt")
                nc.vector.scalar_tensor_tensor(
                    out=chk[:], in0=top8[:], scalar=t_ap, in1=top8[:],
                    op0=Alu.is_ge, op1=Alu.bypass, accum_out=cnt_sb[:],
                )
                chk8 = spool.tile([P, n_chunks], f32, tag="chk8")
                susp_sb = spool.tile([P, 1], f32, tag="susp")
                eighth = top8[:, 7:ncand:8]
                nc.vector.scalar_tensor_tensor(
                    out=chk8[:], in0=eighth, scalar=t_ap, in1=eighth,
                    op0=Alu.is_ge, op1=Alu.bypass, accum_out=susp_sb[:],
                )

                # original feature indices of the selected 64.  indirect_copy
                # shares one interleaved index list per 16-partition group, so
                # gather [P, K*16] and extract the diagonal (s == p%16) where
                # each row's own lookup lands.
                gath = spool.tile([P, K * 16], u32, tag="gath")
                nc.gpsimd.indirect_copy(
                    out=gath[:], data=idxg[:], idxs=cidx_sb[:],
                    i_know_ap_gather_is_preferred=True,
                )
                gathf = spool.tile([P, K * 16], f32, tag="gathf")
                nc.vector.tensor_copy(gathf[:], gath[:])
                prodk = spool.tile([P, K * 16], f32, tag="prodk")
                nc.vector.tensor_tensor(
                    out=prodk[:], in0=gathf[:], in1=diagK[:], op=Alu.mult
                )
                oidxf = spool.tile([P, K], f32, tag="oidxf")
                nc.vector.reduce_sum(
                    oidxf[:],
                    prodk[:].rearrange("p (k s) -> p k s", s=16),
                    axis=mybir.AxisListType.X,
                )
                oidx_sb = spool.tile([P, K], u32, tag="oidx")
                nc.vector.tensor_copy(oidx_sb[:], oidxf[:])

                nc.sync.dma_start(vals_o[bs, :], vals_sb[:])
                nc.sync.dma_start(oidx_o[bs, :], oidx_sb[:])
                nc.sync.dma_start(cnt_o[bs, :], cnt_sb[:])
                nc.sync.dma_start(susp_o[bs, :], susp_sb[:])

                # ---- C: threshold-mask resident feats into sparse ----
                for c in range(n_chunks):
                    cs = slice(c * CHUNK, (c + 1) * CHUNK)
                    sch = opool.tile([P, CHUNK], f32, tag="sch")
                    nc.vector.scalar_tensor_tensor(
                        out=sch[:], in0=feats[:, cs], scalar=t_ap,
                        in1=feats[:, cs], op0=Alu.is_ge, op1=Alu.mult,
                    )
                    nc.sync.dma_start(sparse[bs, cs], sch[:])

                # ---- D: decode via indirect row gathers + per-row MAC ----
                acc = spool.tile([P, dim], f32, tag="acc")
                nc.vector.memset(acc[:], 0.0)
                for g in range(K):
                    gt = gpool.tile([P, dim], f32, tag="gt")
                    nc.gpsimd.indirect_dma_start(
                        out=gt[:],
                        out_offset=None,
                        in_=wdecT.ap(),
                        in_offset=IndirectOffsetOnAxis(
                            ap=oidx_sb[:, g : g + 1], axis=0
                        ),
                    )
                    nc.vector.scalar_tensor_tensor(
                        out=acc[:], in0=gt[:], scalar=vals_sb[:, g : g + 1],
                        in1=acc[:], op0=Alu.mult, op1=Alu.add,
                    )
                nc.sync.dma_start(recon[bs, :], acc[:])

    nc.compile()
    return nc


_NC_CACHE = {}


def _get_kernel(bpc, dim, dict_size):
    key = (bpc, dim, dict_size)
    if key not in _NC_CACHE:
        _NC_CACHE[key] = build_kernel(bpc, dim, dict_size)
    return _NC_CACHE[key]


def _host_topk_row(feats_row, k=K):
    """jax.lax.top_k semantics: descending values, ties broken by lower index."""
    idx = np.argsort(-feats_row, kind="stable")[:k]
    return feats_row[idx], idx


def _repair_rows(rows, x, W_enc, W_dec, sparse, recon, vals_all):
    for r in rows:
        feats_row = np.maximum(x[r] @ W_enc.T, 0.0).astype(np.float32)
        v, i = _host_topk_row(feats_row)
        srow = np.zeros(W_enc.shape[0], dtype=np.float32)
        srow[i] = v
        sparse[r] = srow
        recon[r] = (W_dec[:, i] * v[None, :]).sum(axis=1).astype(np.float32)
        vals_all[r] = v


LAST_EXEC_NS = None
LAST_RESULTS = None


def kernel(x, W_enc, W_dec, _trace=False):
    global LAST_EXEC_NS, LAST_RESULTS
    x = np.asarray(x, dtype=np.float32)
    W_enc = np.asarray(W_enc, dtype=np.float32)
    W_dec = np.asarray(W_dec, dtype=np.float32)
    batch, dim = x.shape
    dict_size = W_enc.shape[0]
    bpc = batch // N_CORES

    wencT = np.ascontiguousarray(W_enc.T)
    wdecT = np.ascontiguousarray(W_dec.T)
    pp, ss = np.arange(P)[:, None], np.tile(np.arange(16), K)[None, :]
    diagk = ((pp % 16) == ss).astype(np.float32)
    in_maps = []
    for c in range(N_CORES):
        xs = x[c * bpc : (c + 1) * bpc]
        in_maps.append(
            {
                "xt": np.ascontiguousarray(xs.T),
                "wencT": wencT,
                "wdecT": wdecT,
                "diagk": diagk,
            }
        )

    nc = _get_kernel(bpc, dim, dict_size)
    res = run_bass_kernel_spmd(
        nc, in_maps, core_ids=list(range(N_CORES)), trace=_trace
    )
    LAST_EXEC_NS = res.exec_time_ns
    LAST_RESULTS = res

    sparse = np.concatenate([r["sparse"] for r in res.results], axis=0)
    recon = np.concatenate([r["recon"] for r in res.results], axis=0)
    vals_all = np.concatenate([r["vals"] for r in res.results], axis=0)
    oidx_all = np.concatenate([r["oidx"] for r in res.results], axis=0)
    cnt = np.concatenate([r["cnt"] for r in res.results], axis=0)[:, 0]
    susp = np.concatenate([r["susp"] for r in res.results], axis=0)[:, 0]

    # exact repair of rows with boundary ties / chunk overflow / dup values
    bad = (cnt != float(K)) | (susp != 0.0)
    srt = np.sort(oidx_all, axis=1)
    dup = (srt[:, 1:] == srt[:, :-1]).any(axis=1)
    bad_rows = np.nonzero(bad | dup)[0]
    if len(bad_rows):
        _repair_rows(bad_rows, x, W_enc, W_dec, sparse, recon, vals_all)

    l0 = np.float32((vals_all > 0).sum(axis=1).mean())
    return recon, sparse, l0


if __name__ == "__main__":
    build_kernel(bpc=128, dim=256, dict_size=2048)
    print("build ok")


# revision 30
# speedup vs baseline: 1.1983x; 1.0569x over previous
"""Trainium2 Bass kernel for the EncoderSAE problem.

Computes, for x (4096, 1024), W_enc (32768, 1024), W_dec (1024, 32768):
    feats = relu(x @ W_enc.T)
    vals, idx = top_k(feats, 64)
    sparse = scatter(vals at idx)            # (4096, 32768)
    reconstructed = sparse @ W_dec.T          # (4096, 1024)
    l0 = mean(count(sparse > 0, axis=1))

Strategy: data-parallel over the batch on 8 NeuronCores (512 rows each).
Per core, rows are processed in 4 blocks of 128 (one per partition).
Block-outer loop: a block's full feature row (128 x 32768 fp32, 16 MB)
stays resident in SBUF while its 64 encode chunks (512 features each,
fp32 PE matmuls) stream through PSUM.  Top-64 per row is computed
exactly from per-chunk top-8 candidates (DVE max8) + 8 rounds of
max8/match_replace over the 512 candidates; the 64th value is a per-row
threshold used to mask the resident feats into the sparse output.
Decode gathers the 64 selected decoder rows per batch row with indirect
DMA and accumulates them with per-partition-scalar multiply-add on the
vector engine.  Rare exact-tie/chunk-overflow rows are detected
on-device (cnt/susp outputs) and repaired exactly on host.
"""

import sys

if "/opt/trn_rl_repo" not in sys.path:
    sys.path.insert(0, "/opt/trn_rl_repo")

import numpy as np

import concourse.bass as bass
import concourse.mybir as mybir
import concourse.tile as tile
from concourse import bacc
from concourse.bass import IndirectOffsetOnAxis
from concourse.bass_utils import run_bass_kernel_spmd

# Problem constants
INPUT_DIM = 1024
DICT_SIZE = 32768
K = 64
BATCH = 4096
N_CORES = 8

P = 128  # partitions
CHUNK = 512  # features per encode chunk


def build_kernel(
    bpc: int = BATCH // N_CORES,  # batch rows per core
    dim: int = INPUT_DIM,
    dict_size: int = DICT_SIZE,
):
    n_blocks = bpc // P
    n_chunks = dict_size // CHUNK
    n_k = dim // P  # contraction tiles
    f32 = mybir.dt.float32
    u32 = mybir.dt.uint32
    u16 = mybir.dt.uint16

    nc = bacc.Bacc("TRN2", target_bir_lowering=False, num_devices=N_CORES)

    xt = nc.dram_tensor("xt", [dim, bpc], f32, kind="ExternalInput")
    wencT = nc.dram_tensor("wencT", [dim, dict_size], f32, kind="ExternalInput")
    wdecT = nc.dram_tensor("wdecT", [dict_size, dim], f32, kind="ExternalInput")
    diagk = nc.dram_tensor("diagk", [P, K * 16], f32, kind="ExternalInput")

    sparse = nc.dram_tensor("sparse", [bpc, dict_size], f32, kind="ExternalOutput")
    recon = nc.dram_tensor("recon", [bpc, dim], f32, kind="ExternalOutput")
    vals_o = nc.dram_tensor("vals", [bpc, K], f32, kind="ExternalOutput")
    oidx_o = nc.dram_tensor("oidx", [bpc, K], u32, kind="ExternalOutput")
    cnt_o = nc.dram_tensor("cnt", [bpc, 1], f32, kind="ExternalOutput")
    susp_o = nc.dram_tensor("susp", [bpc, 1], f32, kind="ExternalOutput")

    Relu = mybir.ActivationFunctionType.Relu
    Alu = mybir.AluOpType
    ncand = n_chunks * 8

    with tile.TileContext(nc) as tc:
        with (
            tc.tile_pool(name="persist", bufs=1) as persist,
            tc.tile_pool(name="xpool", bufs=2) as xpool,
            tc.tile_pool(name="wpool", bufs=8) as wpool,
            tc.tile_pool(name="fpool", bufs=66) as fpool,
            tc.tile_pool(name="cpool", bufs=2) as cpool,
            tc.tile_pool(name="spool", bufs=1) as spool,
            tc.tile_pool(name="opool", bufs=2) as opool,
            tc.tile_pool(name="gpool", bufs=3) as gpool,
            tc.tile_pool(name="psum", bufs=2, space="PSUM") as pspool,
        ):
            # constant mask for diagonal extraction of the 16-partition-shared
            # indirect_copy gather: diagK[p, j*16+s] = (s == p%16)
            diagK = persist.tile([P, K * 16], f32)
            nc.sync.dma_start(diagK[:], diagk[:])

            for b in range(n_blocks):
                bs = slice(b * P, (b + 1) * P)

                # x block, d-tile on partitions: lhsT tiles [128d, 128b]
                xt_sb = xpool.tile([P, n_k, P], f32, tag="xt")
                for k in range(n_k):
                    nc.sync.dma_start(xt_sb[:, k, :], xt[k * P : (k + 1) * P, bs])

                feats_t = []
                top8 = cpool.tile([P, ncand], f32, tag="top8")
                idxg = cpool.tile([P, ncand], u32, tag="idxg")

                # ---- A: encode chunks; feats stay resident ----
                for c in range(n_chunks):
                    wk = []
                    for k in range(n_k):
                        w_sb = wpool.tile([P, CHUNK], f32, tag="w", name=f"w_{b}_{c}_{k}")
                        nc.sync.dma_start(
                            w_sb[:],
                            wencT[k * P : (k + 1) * P, c * CHUNK : (c + 1) * CHUNK],
                        )
                        wk.append(w_sb)
                    ps = pspool.tile([P, CHUNK], f32, tag="enc")
                    for k in range(n_k):
                        nc.tensor.matmul(
                            out=ps[:],
                            lhsT=xt_sb[:, k, :],
                            rhs=wk[k][:],
                            start=(k == 0),
                            stop=(k == n_k - 1),
                        )
                    fcht = fpool.tile([P, CHUNK], f32, tag="feats",
                                      name=f"f_{b}_{c}")
                    feats_t.append(fcht)
                    fch = fcht[:]
                    nc.scalar.activation(fch, ps[:], Relu)
                    t8 = top8[:, c * 8 : (c + 1) * 8]
                    ig = idxg[:, c * 8 : (c + 1) * 8]
                    nc.vector.max(out=t8, in_=fch)
                    nc.vector.max_index(out=ig, in_max=t8, in_values=fch)
                    if c > 0:
                        # convert chunk-local index to global feature index
                        nc.vector.tensor_scalar_add(ig, ig, float(c * CHUNK))

                # ---- B: top-64 of the 512 candidates ----
                work = spool.tile([P, ncand], f32, tag="work")
                nc.vector.tensor_copy(work[:], top8[:])
                vals_sb = spool.tile([P, K], f32, tag="vals")
                cidx_sb = spool.tile([P, K], u16, tag="cidx")
                for r in range(K // 8):
                    m8 = vals_sb[:, r * 8 : (r + 1) * 8]
                    nc.vector.max(out=m8, in_=work[:])
                    nc.vector.max_index(
                        out=cidx_sb[:, r * 8 : (r + 1) * 8],
                        in_max=m8,
                        in_values=work[:],
                    )
                    nc.vector.match_replace(
                        out=work[:], in_to_replace=m8, in_values=work[:],
                        imm_value=-1.0,
                    )
                t_ap = vals_sb[:, K - 1 : K]

                # exactness checks: cnt = #candidates >= T (must be 64);
                # susp = #chunks whose 8th max >= T (must be 0)
                chk = spool.tile([P, ncand], f32, tag="chk")
                cnt_sb = spool.tile([P, 1], f32, tag="cnt")
                nc.vector.scalar_tensor_tensor(
                    out=chk[:], in0=top8[:], scalar=t_ap, in1=top8[:],
                    op0=Alu.is_ge, op1=Alu.bypass, accum_out=cnt_sb[:],
                )
                chk8 = spool.tile([P, n_chunks], f32, tag="chk8")
                susp_sb = spool.tile([P, 1], f32, tag="susp")
                eighth = top8[:, 7:ncand:8]
                nc.vector.scalar_tensor_tensor(
                    out=chk8[:], in0=eighth, scalar=t_ap, in1=eighth,
                    op0=Alu.is_ge, op1=Alu.bypass, accum_out=susp_sb[:],
                )

                # original feature indices of the selected 64.  indirect_copy
                # shares one interleaved index list per 16-partition group, so
                # gather [P, K*16] and extract the diagonal (s == p%16) where
                # each row's own lookup lands.
                gath = spool.tile([P, K * 16], u32, tag="gath")
                nc.gpsimd.indirect_copy(
                    out=gath[:], data=idxg[:], idxs=cidx_sb[:],
                    i_know_ap_gather_is_preferred=True,
                )
                gathf = spool.tile([P, K * 16], f32, tag="gathf")
                nc.vector.tensor_copy(gathf[:], gath[:])
                prodk = spool.tile([P, K * 16], f32, tag="prodk")
                nc.vector.tensor_tensor(
                    out=prodk[:], in0=gathf[:], in1=diagK[:], op=Alu.mult
                )
                oidxf = spool.tile([P, K], f32, tag="oidxf")
                nc.vector.reduce_sum(
                    oidxf[:],
                    prodk[:].rearrange("p (k s) -> p k s", s=16),
                    axis=mybir.AxisListType.X,
                )
                oidx_sb = spool.tile([P, K], u32, tag="oidx")
                nc.vector.tensor_copy(oidx_sb[:], oidxf[:])

                nc.sync.dma_start(vals_o[bs, :], vals_sb[:])
                nc.sync.dma_start(oidx_o[bs, :], oidx_sb[:])
                nc.sync.dma_start(cnt_o[bs, :], cnt_sb[:])
                nc.sync.dma_start(susp_o[bs, :], susp_sb[:])

                # ---- C: threshold-mask resident feats into sparse ----
                # split between vector and gpsimd so it overlaps the next
                # block's encode-side DVE work
                for c in range(n_chunks):
                    cs = slice(c * CHUNK, (c + 1) * CHUNK)
                    sch = opool.tile([P, CHUNK], f32, tag="sch")
                    nc.vector.scalar_tensor_tensor(
                        out=sch[:], in0=feats_t[c][:], scalar=t_ap,
                        in1=feats_t[c][:], op0=Alu.is_ge, op1=Alu.mult,
                    )
                    nc.sync.dma_start(sparse[bs, cs], sch[:])

                # ---- D: decode via indirect row gathers + per-row MAC ----
                acc = spool.tile([P, dim], f32, tag="acc")
                nc.vector.memset(acc[:], 0.0)
                for g in range(K):
                    gt = gpool.tile([P, dim], f32, tag="gt")
                    nc.gpsimd.indirect_dma_start(
                        out=gt[:],
                        out_offset=None,
                        in_=wdecT.ap(),
                        in_offset=IndirectOffsetOnAxis(
                            ap=oidx_sb[:, g : g + 1], axis=0
                        ),
                    )
                    nc.vector.scalar_tensor_tensor(
                        out=acc[:], in0=gt[:], scalar=vals_sb[:, g : g + 1],
                        in1=acc[:], op0=Alu.mult, op1=Alu.add,
                    )
                nc.sync.dma_start(recon[bs, :], acc[:])

    nc.compile()
    return nc


_NC_CACHE = {}


def _get_kernel(bpc, dim, dict_size):
    key = (bpc, dim, dict_size)
    if key not in _NC_CACHE:
        _NC_CACHE[key] = build_kernel(bpc, dim, dict_size)
    return _NC_CACHE[key]


def _host_topk_row(feats_row, k=K):
    """jax.lax.top_k semantics: descending values, ties broken by lower index."""
    idx = np.argsort(-feats_row, kind="stable")[:k]
    return feats_row[idx], idx


def _repair_rows(rows, x, W_enc, W_dec, sparse, recon, vals_all):
    for r in rows:
        feats_row = np.maximum(x[r] @ W_enc.T, 0.0).astype(np.float32)
        v, i = _host_topk_row(feats_row)
        srow = np.zeros(W_enc.shape[0], dtype=np.float32)
        srow[i] = v
        sparse[r] = srow
        recon[r] = (W_dec[:, i] * v[None, :]).sum(axis=1).astype(np.float32)
        vals_all[r] = v


LAST_EXEC_NS = None
LAST_RESULTS = None


def kernel(x, W_enc, W_dec, _trace=False):
    global LAST_EXEC_NS, LAST_RESULTS
    x = np.asarray(x, dtype=np.float32)
    W_enc = np.asarray(W_enc, dtype=np.float32)
    W_dec = np.asarray(W_dec, dtype=np.float32)
    batch, dim = x.shape
    dict_size = W_enc.shape[0]
    bpc = batch // N_CORES

    wencT = np.ascontiguousarray(W_enc.T)
    wdecT = np.ascontiguousarray(W_dec.T)
    pp, ss = np.arange(P)[:, None], np.tile(np.arange(16), K)[None, :]
    diagk = ((pp % 16) == ss).astype(np.float32)
    in_maps = []
    for c in range(N_CORES):
        xs = x[c * bpc : (c + 1) * bpc]
        in_maps.append(
            {
                "xt": np.ascontiguousarray(xs.T),
                "wencT": wencT,
                "wdecT": wdecT,
                "diagk": diagk,
            }
        )

    nc = _get_kernel(bpc, dim, dict_size)
    res = run_bass_kernel_spmd(
        nc, in_maps, core_ids=list(range(N_CORES)), trace=_trace
    )
    LAST_EXEC_NS = res.exec_time_ns
    LAST_RESULTS = res

    sparse = np.concatenate([r["sparse"] for r in res.results], axis=0)
    recon = np.concatenate([r["recon"] for r in res.results], axis=0)
    vals_all = np.concatenate([r["vals"] for r in res.results], axis=0)
    oidx_all = np.concatenate([r["oidx"] for r in res.results], axis=0)
    cnt = np.concatenate([r["cnt"] for r in res.results], axis=0)[:, 0]
    susp = np.concatenate([r["susp"] for r in res.results], axis=0)[:, 0]

    # exact repair of rows with boundary ties / chunk overflow / dup values
    bad = (cnt != float(K)) | (susp != 0.0)
    srt = np.sort(oidx_all, axis=1)
    dup = (srt[:, 1:] == srt[:, :-1]).any(axis=1)
    bad_rows = np.nonzero(bad | dup)[0]
    if len(bad_rows):
        _repair_rows(bad_rows, x, W_enc, W_dec, sparse, recon, vals_all)

    l0 = np.float32((vals_all > 0).sum(axis=1).mean())
    return recon, sparse, l0


if __name__ == "__main__":
    build_kernel(bpc=128, dim=256, dict_size=2048)
    print("build ok")
